# revision 29
# baseline (speedup 1.0000x reference)
"""Sparse-attention (entity_mention_select) Trainium2 kernel.

Per entity b: q = relation_matrix[label_b]; scores = node_b @ q over the
active nodes (edge_weight==1); softmax; out_b = softmax(scores) @ node_b.

Strategy (v4):
  - Host gathers only the ACTIVE nodes per entity (~50% of N), casts to
    bf16, and packs them into a [128, C_k*258] tile per position with 258
    columns per slot: 256 features, a 1.0 "denominator" column, and a 0.0
    alignment pad.  HBM traffic drops ~3.4x vs the f32 full-N baseline.
  - Entities are sorted by active count and dealt round-robin to the 8
    cores, so all cores share one chunk schedule C[64] (SPMD).
  - Pad slots are all-zero rows: score==0 so em=exp(0)=1, but the zero
    denominator column knocks their contribution out of the sum, and the
    zero feature columns knock them out of the numerator.  The softmax
    denominator therefore lands exactly in PSUM column 256 of each
    position's out-matmul accumulation - no mask, no correction pass.
  - Device pipeline per position k (software-pipelined so every engine's
    queue stays dense):
      SB Pool: broadcast the q pair to 128 partitions (partition_broadcast)
      S2 DVE : C_k dot-product chunks (bf16) with accum -> scores
      S3 ACT : exp(scores) -> em (bf16)
      S4 PE  : C_k matmuls em^T @ node[:,0:258] -> out row + den (PSUM)
      group  : DVE reciprocal over the 4 strided den cells
      S6 ACT : per-position scaled PSUM->SBUF copy (scale = 1/den)
"""

import sys

for _p in ("/opt/trn_rl_repo", "/root/.axon_site/_ro/trn_rl_repo"):
    if _p not in sys.path:
        sys.path.append(_p)

import numpy as np
import ml_dtypes
from contextlib import ExitStack

import concourse.tile as tile
from concourse import bacc, mybir
from concourse.bass_utils import run_bass_kernel_spmd

F32 = mybir.dt.float32
BF16 = mybir.dt.bfloat16
ALU = mybir.AluOpType
ACTF = mybir.ActivationFunctionType

B, N, D, R = 512, 1024, 256, 100
DP = D + 2             # slot columns: 256 features + 1.0 den col + 0.0 pad
NCORES = 8
BPC = B // NCORES      # 64 entities (positions) per core
GRP = 2                # positions per output group (PSUM tile)
POS_PER_DMA = 4        # positions per node DMA
NDMA = BPC // POS_PER_DMA
OSTR = 512             # f32 stride between positions inside a group PSUM tile


# ---------------------------------------------------------------------------
# schedule plan (host, data-dependent)
# ---------------------------------------------------------------------------


def make_plan(edge_weight):
    cnt = np.asarray(edge_weight).sum(axis=1).astype(np.int64)  # [B]
    order = np.argsort(cnt, kind="stable")                      # ascending
    perm = order.reshape(BPC, NCORES)                           # perm[k, c]
    pos_max = cnt[perm].max(axis=1)                             # [BPC]
    C = np.maximum(1, -(-pos_max // 128)).astype(np.int64)      # ceil/128
    return perm, tuple(int(c) for c in C), cnt


# ---------------------------------------------------------------------------
# device kernel
# ---------------------------------------------------------------------------


def build_tile_kernel(tc, outs, ins, C):
    nc = tc.nc
    node = ins["node"]          # [128, TOT*DP] bf16
    qflat = ins["qflat"]        # [1, BPC*D] bf16 (q row per position)
    out = outs["out"]           # [1, BPC*D] f32

    C = list(C)
    coff = np.concatenate([[0], np.cumsum(C)]).astype(int)  # chunk offsets
    TOTC = int(coff[-1])                                    # total chunks

    # node DMA groups: small leading groups so compute starts early, then
    # POS_PER_DMA-position groups.  dma_pos[i] = first position of DMA i.
    pp = [2, 2] + [POS_PER_DMA] * ((BPC - 4) // POS_PER_DMA)
    dma_pos = np.concatenate([[0], np.cumsum(pp)]).astype(int)
    ndma = len(pp)
    dma_rng = [
        (int(coff[dma_pos[i]]) * DP, int(coff[dma_pos[i + 1]]) * DP)
        for i in range(ndma)
    ]
    max_dma_cols = max(c1 - c0 for c0, c1 in dma_rng)
    pos_dma = np.searchsorted(dma_pos, np.arange(BPC), side="right") - 1

    with ExitStack() as ctx:
        const_pool = ctx.enter_context(tc.tile_pool(name="const", bufs=1))
        node_pool = ctx.enter_context(tc.tile_pool(name="node", bufs=6))
        qb_pool = ctx.enter_context(tc.tile_pool(name="qb", bufs=3))
        scr_pool = ctx.enter_context(tc.tile_pool(name="scr", bufs=2))
        small_pool = ctx.enter_context(tc.tile_pool(name="small", bufs=3))
        work_pool = ctx.enter_context(tc.tile_pool(name="work", bufs=1))
        ps_o = ctx.enter_context(tc.tile_pool(name="ps_o", bufs=3, space="PSUM"))
        ps_qb = ctx.enter_context(tc.tile_pool(name="ps_qb", bufs=2, space="PSUM"))

        # ---------- setup (small inputs first, on the fast HWDGE queues) ----------
        # qflat is a single-partition row; load it in 8 chunks split across
        # both HWDGE queues so the first pairs' q vectors arrive early.
        qflat_sb = const_pool.tile([1, BPC * D], BF16, tag="qflat")
        QCH = BPC * D // 8
        for qc in range(8):
            eng = nc.sync if qc % 2 == 0 else nc.scalar
            eng.dma_start(
                qflat_sb[:1, qc * QCH : (qc + 1) * QCH],
                qflat[:, qc * QCH : (qc + 1) * QCH],
            )
        ones_col_sb = const_pool.tile([1, 128], BF16, tag="ones_col")
        nc.scalar.dma_start(ones_col_sb[:], ins["ones_col"][:, :])

        # ---------- resident work tiles ----------
        scores_all = work_pool.tile([128, TOTC], F32, tag="scores")
        em_all = work_pool.tile([128, TOTC], BF16, tag="em")
        out_all = work_pool.tile([1, BPC * D], F32, tag="out_all")

        # ---------- node DMAs (2 HWDGE queues, interleaved) ----------
        node_sbs = []
        for d in range(ndma):
            c0, c1 = dma_rng[d]
            nsb = node_pool.tile([128, max_dma_cols], BF16, tag="nd")
            eng = nc.sync if d % 2 == 0 else nc.scalar
            eng.dma_start(nsb[:, : c1 - c0], node[:, c0:c1])
            node_sbs.append(nsb)

        def node_chunk(k, j):
            d = int(pos_dma[k])
            base = (coff[k] + j) * DP - dma_rng[d][0]
            return node_sbs[d][:, base : base + DP]

        # ---------- software-pipelined position loop ----------
        qb_sbs = {}
        qb_pss = {}
        o_ps4s = {}
        recips = {}
        LAT = 8
        for t in range(BPC + LAT):
            # S0 (PE): broadcast q pair for positions t, t+1 to 128 partitions
            if t < BPC and t % 2 == 0:
                qb_ps = ps_qb.tile([128, 2 * D], F32, tag="qbps")
                nc.tensor.matmul(
                    qb_ps[:],
                    ones_col_sb[:1, :],
                    qflat_sb[:1, t * D : (t + 2) * D],
                    start=True,
                    stop=True,
                )
                qb_pss[t] = qb_ps

            # S1 (ACT): qb pair PSUM -> SBUF bf16 copy (one iteration later)
            if t >= 1 and (t - 1) % 2 == 0 and t - 1 < BPC:
                p = t - 1
                qb_sb = qb_pool.tile([128, 2 * D], BF16, tag="qb")
                nc.scalar.copy(qb_sb[:], qb_pss.pop(p)[:])
                qb_sbs[p] = qb_sb

            # S3 (ACT): exp for position t-3
            k3 = t - 3
            if 0 <= k3 < BPC:
                sl = slice(int(coff[k3]), int(coff[k3 + 1]))
                nc.scalar.activation(em_all[:, sl], scores_all[:, sl], ACTF.Exp)

            # S2 (DVE): score chunks for position t-2
            k2 = t - 2
            if 0 <= k2 < BPC:
                qb = qb_sbs[k2 - (k2 % 2)][:, (k2 % 2) * D : (k2 % 2 + 1) * D]
                for j in range(C[k2]):
                    scr = scr_pool.tile([128, D], BF16, tag="scr")
                    nc.vector.scalar_tensor_tensor(
                        scr[:],
                        node_chunk(k2, j)[:, :D],
                        1.0,
                        qb,
                        ALU.mult,
                        ALU.mult,
                        accum_out=scores_all[:, coff[k2] + j : coff[k2] + j + 1],
                    )

            # S4 (PE): out matmuls for position t-4 into the group PSUM tile;
            # the 1.0 column of each slot accumulates the softmax denominator
            # into PSUM column gi*OSTR + 256.
            k4 = t - 4
            if 0 <= k4 < BPC:
                g, gi = divmod(k4, GRP)
                if gi == 0:
                    o_ps4s[g] = ps_o.tile([1, GRP * OSTR], F32, tag="o", name="o_ps4")
                o_ps4 = o_ps4s[g]
                for j in range(C[k4]):
                    nc.tensor.matmul(
                        o_ps4[:1, gi * OSTR : gi * OSTR + DP],
                        em_all[:, coff[k4] + j : coff[k4] + j + 1],
                        node_chunk(k4, j),
                        start=(j == 0),
                        stop=(j == C[k4] - 1),
                    )
                if gi == GRP - 1:
                    recip = small_pool.tile([1, GRP], F32, tag="recip")
                    nc.vector.reciprocal(
                        recip[:], o_ps4[:1, D : GRP * OSTR : OSTR]
                    )
                    recips[g] = recip

            # S6 (ACT): per-position scaled PSUM->SBUF copies for group t-8
            k6 = t - 8
            if 0 <= k6 < BPC and k6 % GRP == 0:
                g = k6 // GRP
                recip = recips.pop(g)
                o_ps4 = o_ps4s.pop(g)
                for gi in range(GRP):
                    kk = g * GRP + gi
                    nc.scalar.activation(
                        out_all[:1, kk * D : (kk + 1) * D],
                        o_ps4[:1, gi * OSTR : gi * OSTR + D],
                        ACTF.Copy,
                        scale=recip[:1, gi : gi + 1],
                    )

            # chunked output DMA once each 16-position span is scaled
            k7 = t - 8
            if 0 <= k7 < BPC and k7 % 16 == 16 - GRP:
                m = k7 // 16
                nc.scalar.dma_start(
                    out[:, m * 16 * D : (m + 1) * 16 * D],
                    out_all[:1, m * 16 * D : (m + 1) * 16 * D],
                )


# ---------------------------------------------------------------------------
# host-side driver
# ---------------------------------------------------------------------------

_CACHE = {}


def build_nc(C, loop_trip=None):
    C = tuple(C)
    TOT = int(sum(C)) * DP
    nc = bacc.Bacc(
        "TRN2",
        target_bir_lowering=False,
        debug=False,
        enable_asserts=False,
        num_devices=NCORES,
    )
    ins = {
        "node": nc.dram_tensor("node", [128, TOT], BF16, kind="ExternalInput").ap(),
        "qflat": nc.dram_tensor("qflat", [1, BPC * D], BF16, kind="ExternalInput").ap(),
        "ones_col": nc.dram_tensor("ones_col", [1, 128], BF16, kind="ExternalInput").ap(),
    }
    outs = {"out": nc.dram_tensor("out", [1, BPC * D], F32, kind="ExternalOutput").ap()}
    with tile.TileContext(nc) as tc:
        if loop_trip is None:
            build_tile_kernel(tc, outs, ins, C)
        else:
            with tc.For_i(0, loop_trip, 1):
                build_tile_kernel(tc, outs, ins, C)
    nc.compile()
    return nc


def _get_nc(C):
    C = tuple(C)
    if C not in _CACHE:
        _CACHE[C] = build_nc(C)
    return _CACHE[C]


def make_in_maps(node_feature, edge_weight, relation_label, relation_matrix,
                 plan=None):
    node_feature = np.asarray(node_feature)
    edge_weight = np.asarray(edge_weight)
    relation_label = np.asarray(relation_label)
    relation_matrix = np.asarray(relation_matrix, dtype=np.float32)
    if plan is None:
        plan = make_plan(edge_weight)
    perm, C, cnt = plan
    coff = np.concatenate([[0], np.cumsum(C)]).astype(int)
    TOT = int(coff[-1]) * DP

    qmat = relation_matrix.astype(ml_dtypes.bfloat16)
    ones_col = np.ones((1, 128), ml_dtypes.bfloat16)

    in_maps = []
    for c in range(NCORES):
        node_packed = np.zeros((128, TOT), ml_dtypes.bfloat16)
        qflat = np.zeros((1, BPC * D), ml_dtypes.bfloat16)
        for k in range(BPC):
            e = int(perm[k, c])
            ck = int(C[k])
            nslots = ck * 128
            m = int(cnt[e])
            buf = np.zeros((nslots, DP), ml_dtypes.bfloat16)
            buf[:m, :D] = node_feature[e][edge_weight[e] == 1]
            buf[:m, D] = 1.0
            node_packed[:, coff[k] * DP : coff[k + 1] * DP] = buf.reshape(128, ck * DP)
            qflat[0, k * D : (k + 1) * D] = qmat[int(relation_label[e])]
        in_maps.append(
            {"node": node_packed, "qflat": qflat, "ones_col": ones_col}
        )
    return in_maps


def run(node_feature, edge_weight, relation_label, relation_matrix, trace=False):
    plan = make_plan(np.asarray(edge_weight))
    perm, C, cnt = plan
    nc = _get_nc(C)
    in_maps = make_in_maps(
        node_feature, edge_weight, relation_label, relation_matrix, plan=plan
    )
    res = run_bass_kernel_spmd(nc, in_maps, core_ids=list(range(NCORES)), trace=trace)
    out = np.zeros((B, D), np.float32)
    for c in range(NCORES):
        out[perm[:, c]] = np.asarray(res.results[c]["out"], dtype=np.float32).reshape(
            BPC, D
        )
    return out, res


def kernel(node_feature, edge_weight, relation_label, relation_matrix):
    out, _ = run(node_feature, edge_weight, relation_label, relation_matrix)
    return out


# ---------------------------------------------------------------------------
# wall-clock timing helper (no NTFF profiling available under this axon setup)
# ---------------------------------------------------------------------------


def make_timed_runner(nc, in_maps):
    """Build a jitted 8-core runner with inputs resident on device.

    Returns (call, out_names): `call()` executes once, blocking, and returns
    the jax output arrays. Mirrors bass2jax.run_bass_via_pjrt's multi-core
    branch, but keeps the big inputs on device across calls so repeated calls
    time [dispatch + kernel exec] only.
    """
    import jax
    from jax.sharding import Mesh, PartitionSpec
    from jax.experimental.shard_map import shard_map
    from concourse import bass2jax as b2j
    from concourse import mybir as _mb

    b2j.install_neuronx_cc_hook()
    n_cores = len(in_maps)

    partition_name = nc.partition_id_tensor.name if nc.partition_id_tensor else None
    in_names, out_names, out_avals, zero_outs = [], [], [], []
    for alloc in nc.m.functions[0].allocations:
        if not isinstance(alloc, _mb.MemoryLocationSet):
            continue
        name = alloc.memorylocations[0].name
        if alloc.kind == "ExternalInput":
            if name != partition_name:
                in_names.append(name)
        elif alloc.kind == "ExternalOutput":
            out_names.append(name)
            shape = tuple(alloc.tensor_shape)
            dtype = _mb.dt.np(alloc.dtype)
            out_avals.append(jax.core.ShapedArray(shape, dtype))
            zero_outs.append(np.zeros(shape, dtype))
    n_params = len(in_names)
    all_in_names = in_names + out_names
    if partition_name is not None:
        all_in_names.append(partition_name)

    def _body(*args):
        operands = list(args)
        if partition_name is not None:
            operands.append(b2j.partition_id_tensor())
        outs = b2j._bass_exec_p.bind(
            *operands,
            out_avals=tuple(out_avals),
            in_names=tuple(all_in_names),
            out_names=tuple(out_names),
            lowering_input_output_aliases=(),
            sim_require_finite=True,
            sim_require_nnan=True,
            nc=nc,
        )
        return tuple(outs)

    devices = jax.devices()[:n_cores]
    mesh = Mesh(np.asarray(devices), ("core",))
    in_specs = (PartitionSpec("core"),) * (n_params + len(out_names))
    out_specs = (PartitionSpec("core"),) * len(out_names)
    donate = tuple(range(n_params, n_params + len(out_names)))
    sharded = jax.jit(
        shard_map(
            _body, mesh=mesh, in_specs=in_specs, out_specs=out_specs, check_rep=False
        ),
        donate_argnums=donate,
        keep_unused=True,
    )

    sharding = jax.sharding.NamedSharding(mesh, PartitionSpec("core"))
    dev_in = [
        jax.device_put(
            np.concatenate([np.asarray(m[name]) for m in in_maps], axis=0), sharding
        )
        for name in in_names
    ]

    def call():
        zeros = [np.zeros((n_cores * z.shape[0], *z.shape[1:]), z.dtype) for z in zero_outs]
        outs = sharded(*dev_in, *zeros)
        jax.block_until_ready(outs)
        return outs

    return call, out_names


# revision 30
# speedup vs baseline: 1.3379x; 1.3379x over previous
"""Sparse-attention (entity_mention_select) Trainium2 kernel.

Per entity b: q = relation_matrix[label_b]; scores = node_b @ q over the
active nodes (edge_weight==1); softmax; out_b = softmax(scores) @ node_b.

Strategy (v2):
  - Host gathers only the ACTIVE nodes per entity (~50% of N), casts to
    bf16, and packs them into a [128, C_k*256] tile per entity (slot
    s = p*C_k + j).  HBM traffic drops ~3.5x vs the f32 full-N baseline.
  - Entities are sorted by active count and dealt round-robin to the 8
    cores, so all cores share one chunk schedule C[64] (SPMD).
  - Pad slots are zero rows: score==0 exactly, so each pad contributes
    exactly exp(0)=1 to the softmax denominator; the host passes the pad
    counts and the kernel subtracts them inside the denominator matmul.
  - Device pipeline per position k (software-pipelined with stage
    offsets so each engine's queue stays dense):
      S0 PE : broadcast q pair to 128 partitions (1 matmul / 2 entities)
      S1 ACT: PSUM->SBUF bf16 copy of the q pair
      S2 DVE: C_k dot-product chunks (bf16 2x) with accum -> scores
      S3 ACT: exp(scores) -> em (bf16), accum -> esums column
      S4 PE : C_k matmuls em^T @ node -> out row (PSUM)
    plus per group of 8: denominator matmul (+pad fix), reciprocal,
    and a per-partition-scaled PSUM->SBUF copy of the 8 output rows.
"""

import sys

for _p in ("/opt/trn_rl_repo", "/root/.axon_site/_ro/trn_rl_repo"):
    if _p not in sys.path:
        sys.path.append(_p)

import numpy as np
import ml_dtypes
from contextlib import ExitStack

import concourse.tile as tile
from concourse import bacc, mybir
from concourse.bass_utils import run_bass_kernel_spmd

F32 = mybir.dt.float32
BF16 = mybir.dt.bfloat16
ALU = mybir.AluOpType
ACTF = mybir.ActivationFunctionType

B, N, D, R = 512, 1024, 256, 100
NCORES = 8
BPC = B // NCORES      # 64 entities (positions) per core
GRP = 4                # positions per denominator/output group
POS_PER_DMA = 4        # positions per node DMA
NDMA = BPC // POS_PER_DMA


# ---------------------------------------------------------------------------
# schedule plan (host, data-dependent)
# ---------------------------------------------------------------------------


def make_plan(edge_weight):
    cnt = np.asarray(edge_weight).sum(axis=1).astype(np.int64)  # [B]
    order = np.argsort(cnt, kind="stable")                      # ascending
    perm = order.reshape(BPC, NCORES)                           # perm[k, c]
    pos_max = cnt[perm].max(axis=1)                             # [BPC]
    C = np.maximum(1, -(-pos_max // 128)).astype(np.int64)      # ceil/128
    return perm, tuple(int(c) for c in C), cnt


# ---------------------------------------------------------------------------
# device kernel
# ---------------------------------------------------------------------------


def build_tile_kernel(tc, outs, ins, C):
    nc = tc.nc
    node = ins["node"]          # [128, TOT] bf16
    qflat = ins["qflat"]        # [1, BPC*D] bf16 (q row per position)
    pneg = ins["pneg"]          # [1, BPC] f32  (minus pad count per position)
    ones_col = ins["ones_col"]  # [1, 128] bf16
    ones_r = ins["ones_r"]      # [128, 1] f32
    out = outs["out"]           # [1, BPC*D] f32

    C = list(C)
    coff = np.concatenate([[0], np.cumsum(C)]).astype(int)  # chunk offsets
    TOTC = int(coff[-1])                                    # total chunks

    # node DMA groups: positions [4d, 4d+4), column range in chunks*256
    dma_rng = [
        (int(coff[d * POS_PER_DMA]) * D, int(coff[(d + 1) * POS_PER_DMA]) * D)
        for d in range(NDMA)
    ]
    max_dma_cols = max(c1 - c0 for c0, c1 in dma_rng)

    with ExitStack() as ctx:
        const_pool = ctx.enter_context(tc.tile_pool(name="const", bufs=1))
        node_pool = ctx.enter_context(tc.tile_pool(name="node", bufs=6))
        qb_pool = ctx.enter_context(tc.tile_pool(name="qb", bufs=3))
        scr_pool = ctx.enter_context(tc.tile_pool(name="scr", bufs=2))
        small_pool = ctx.enter_context(tc.tile_pool(name="small", bufs=2))
        work_pool = ctx.enter_context(tc.tile_pool(name="work", bufs=1))
        ps_qb = ctx.enter_context(tc.tile_pool(name="ps_qb", bufs=2, space="PSUM"))
        ps_o = ctx.enter_context(tc.tile_pool(name="ps_o", bufs=5, space="PSUM"))
        ps_den = ctx.enter_context(tc.tile_pool(name="ps_den", bufs=1, space="PSUM"))

        # ---------- setup (small inputs on the SWDGE queue) ----------
        qflat_sb = const_pool.tile([1, BPC * D], BF16, tag="qflat")
        nc.gpsimd.dma_start(qflat_sb[:], qflat[:, :])
        pneg_sb = const_pool.tile([1, BPC], F32, tag="pneg")
        nc.gpsimd.dma_start(pneg_sb[:], pneg[:, :])
        ones_col_sb = const_pool.tile([1, 128], BF16, tag="ones_col")
        nc.gpsimd.dma_start(ones_col_sb[:], ones_col[:, :])
        ones_r_sb = const_pool.tile([128, 1], F32, tag="ones_r")
        nc.gpsimd.dma_start(ones_r_sb[:], ones_r[:, :])

        # ---------- resident work tiles ----------
        scores_all = work_pool.tile([128, TOTC], F32, tag="scores")
        em_all = work_pool.tile([128, TOTC], BF16, tag="em")
        esums_all = work_pool.tile([128, BPC], F32, tag="esums")
        out_all = work_pool.tile([1, BPC * D], F32, tag="out_all")

        # ---------- node DMAs (2 HWDGE queues, interleaved) ----------
        node_sbs = []
        for d in range(NDMA):
            c0, c1 = dma_rng[d]
            nsb = node_pool.tile([128, max_dma_cols], BF16, tag="nd")
            eng = nc.sync if d % 2 == 0 else nc.scalar
            eng.dma_start(nsb[:, : c1 - c0], node[:, c0:c1])
            node_sbs.append(nsb)

        def node_chunk(k, j):
            d = k // POS_PER_DMA
            base = (coff[k] + j) * D - dma_rng[d][0]
            return node_sbs[d][:, base : base + D]

        # ---------- software-pipelined position loop ----------
        qb_sbs = {}
        qb_pss = {}
        o_pss = {}
        recips = {}
        LAT = 5
        for t in range(BPC + LAT):
            # S0 (PE): broadcast q pair for positions t, t+1
            if t < BPC and t % 2 == 0:
                qb_ps = ps_qb.tile([128, 512], F32, tag="qbps")
                nc.tensor.matmul(
                    qb_ps[:],
                    ones_col_sb[:1, :],
                    qflat_sb[:1, t * D : (t + 2) * D],
                    start=True,
                    stop=True,
                )
                qb_pss[t] = qb_ps

            # S3 (ACT): exp for position t-4
            k3 = t - 4
            if 0 <= k3 < BPC:
                sl = slice(int(coff[k3]), int(coff[k3 + 1]))
                nc.scalar.activation(
                    em_all[:, sl],
                    scores_all[:, sl],
                    ACTF.Exp,
                    accum_out=esums_all[:, k3 : k3 + 1],
                )
                if k3 % GRP == GRP - 1:
                    g = k3 // GRP
                    rs = slice(g * GRP, (g + 1) * GRP)
                    den_ps = ps_den.tile([1, GRP], F32, tag="den")
                    nc.tensor.matmul(
                        den_ps[:],
                        ones_r_sb[:, :],
                        esums_all[:, rs],
                        start=True,
                        stop=False,
                    )
                    nc.tensor.matmul(
                        den_ps[:],
                        ones_r_sb[:1, :1],
                        pneg_sb[:1, rs],
                        start=False,
                        stop=True,
                    )
                    recip = small_pool.tile([1, GRP], F32, tag="recip")
                    nc.vector.reciprocal(recip[:], den_ps[:])
                    recips[g] = recip

            # S2 (DVE): score chunks for position t-3
            k2 = t - 3
            if 0 <= k2 < BPC:
                qb = qb_sbs[k2 - (k2 % 2)][:, (k2 % 2) * D : (k2 % 2 + 1) * D]
                for j in range(C[k2]):
                    scr = scr_pool.tile([128, D], BF16, tag="scr")
                    nc.vector.scalar_tensor_tensor(
                        scr[:],
                        node_chunk(k2, j),
                        1.0,
                        qb,
                        ALU.mult,
                        ALU.mult,
                        accum_out=scores_all[:, coff[k2] + j : coff[k2] + j + 1],
                    )

            # S1 (ACT): qb pair PSUM -> SBUF bf16 copy for positions t-1, t
            if t >= 1 and (t - 1) % 2 == 0 and t - 1 < BPC:
                p = t - 1
                qb_sb = qb_pool.tile([128, 512], BF16, tag="qb")
                nc.scalar.copy(qb_sb[:], qb_pss.pop(p)[:])
                qb_sbs[p] = qb_sb

            # S4 (PE): output matmuls for position t-5
            k4 = t - 5
            if 0 <= k4 < BPC:
                o_ps = ps_o.tile([1, D], F32, tag="o")
                for j in range(C[k4]):
                    nc.tensor.matmul(
                        o_ps[:],
                        em_all[:, coff[k4] + j : coff[k4] + j + 1],
                        node_chunk(k4, j),
                        start=(j == 0),
                        stop=(j == C[k4] - 1),
                    )
                o_pss[k4] = o_ps
                if k4 % GRP == GRP - 1:
                    g = k4 // GRP
                    recip = recips.pop(g)
                    for kk in range(g * GRP, (g + 1) * GRP):
                        nc.scalar.activation(
                            out_all[:1, kk * D : (kk + 1) * D],
                            o_pss.pop(kk)[:],
                            ACTF.Copy,
                            scale=recip[:1, kk % GRP : kk % GRP + 1],
                        )

        nc.sync.dma_start(out[:, :], out_all[:, :])


# ---------------------------------------------------------------------------
# host-side driver
# ---------------------------------------------------------------------------

_CACHE = {}


def build_nc(C, loop_trip=None):
    C = tuple(C)
    TOT = int(sum(C)) * D
    nc = bacc.Bacc(
        "TRN2",
        target_bir_lowering=False,
        debug=False,
        enable_asserts=False,
        num_devices=NCORES,
    )
    ins = {
        "node": nc.dram_tensor("node", [128, TOT], BF16, kind="ExternalInput").ap(),
        "qflat": nc.dram_tensor("qflat", [1, BPC * D], BF16, kind="ExternalInput").ap(),
        "pneg": nc.dram_tensor("pneg", [1, BPC], F32, kind="ExternalInput").ap(),
        "ones_col": nc.dram_tensor("ones_col", [1, 128], BF16, kind="ExternalInput").ap(),
        "ones_r": nc.dram_tensor("ones_r", [128, 1], F32, kind="ExternalInput").ap(),
    }
    outs = {"out": nc.dram_tensor("out", [1, BPC * D], F32, kind="ExternalOutput").ap()}
    with tile.TileContext(nc) as tc:
        if loop_trip is None:
            build_tile_kernel(tc, outs, ins, C)
        else:
            with tc.For_i(0, loop_trip, 1):
                build_tile_kernel(tc, outs, ins, C)
    nc.compile()
    return nc


def _get_nc(C):
    C = tuple(C)
    if C not in _CACHE:
        _CACHE[C] = build_nc(C)
    return _CACHE[C]


def make_in_maps(node_feature, edge_weight, relation_label, relation_matrix,
                 plan=None):
    node_feature = np.asarray(node_feature)
    edge_weight = np.asarray(edge_weight)
    relation_label = np.asarray(relation_label)
    relation_matrix = np.asarray(relation_matrix, dtype=np.float32)
    if plan is None:
        plan = make_plan(edge_weight)
    perm, C, cnt = plan
    coff = np.concatenate([[0], np.cumsum(C)]).astype(int)
    TOT = int(coff[-1]) * D

    ones_col = np.ones((1, 128), ml_dtypes.bfloat16)
    ones_r = np.ones((128, 1), np.float32)
    qmat = relation_matrix.astype(ml_dtypes.bfloat16)

    in_maps = []
    for c in range(NCORES):
        node_packed = np.zeros((128, TOT), ml_dtypes.bfloat16)
        qflat = np.zeros((1, BPC * D), ml_dtypes.bfloat16)
        pneg = np.zeros((1, BPC), np.float32)
        for k in range(BPC):
            e = int(perm[k, c])
            ck = int(C[k])
            nslots = ck * 128
            m = int(cnt[e])
            buf = np.zeros((nslots, D), ml_dtypes.bfloat16)
            buf[:m] = node_feature[e][edge_weight[e] == 1]
            node_packed[:, coff[k] * D : coff[k + 1] * D] = buf.reshape(128, ck * D)
            qflat[0, k * D : (k + 1) * D] = qmat[int(relation_label[e])]
            pneg[0, k] = -(nslots - m)
        in_maps.append(
            {
                "node": node_packed,
                "qflat": qflat,
                "pneg": pneg,
                "ones_col": ones_col,
                "ones_r": ones_r,
            }
        )
    return in_maps


def run(node_feature, edge_weight, relation_label, relation_matrix, trace=False):
    plan = make_plan(np.asarray(edge_weight))
    perm, C, cnt = plan
    nc = _get_nc(C)
    in_maps = make_in_maps(
        node_feature, edge_weight, relation_label, relation_matrix, plan=plan
    )
    res = run_bass_kernel_spmd(nc, in_maps, core_ids=list(range(NCORES)), trace=trace)
    out = np.zeros((B, D), np.float32)
    for c in range(NCORES):
        out[perm[:, c]] = np.asarray(res.results[c]["out"], dtype=np.float32).reshape(
            BPC, D
        )
    return out, res


def kernel(node_feature, edge_weight, relation_label, relation_matrix):
    out, _ = run(node_feature, edge_weight, relation_label, relation_matrix)
    return out


# ---------------------------------------------------------------------------
# wall-clock timing helper (no NTFF profiling available under this axon setup)
# ---------------------------------------------------------------------------


def make_timed_runner(nc, in_maps):
    """Build a jitted 8-core runner with inputs resident on device.

    Returns (call, out_names): `call()` executes once, blocking, and returns
    the jax output arrays. Mirrors bass2jax.run_bass_via_pjrt's multi-core
    branch, but keeps the big inputs on device across calls so repeated calls
    time [dispatch + kernel exec] only.
    """
    import jax
    from jax.sharding import Mesh, PartitionSpec
    from jax.experimental.shard_map import shard_map
    from concourse import bass2jax as b2j
    from concourse import mybir as _mb

    b2j.install_neuronx_cc_hook()
    n_cores = len(in_maps)

    partition_name = nc.partition_id_tensor.name if nc.partition_id_tensor else None
    in_names, out_names, out_avals, zero_outs = [], [], [], []
    for alloc in nc.m.functions[0].allocations:
        if not isinstance(alloc, _mb.MemoryLocationSet):
            continue
        name = alloc.memorylocations[0].name
        if alloc.kind == "ExternalInput":
            if name != partition_name:
                in_names.append(name)
        elif alloc.kind == "ExternalOutput":
            out_names.append(name)
            shape = tuple(alloc.tensor_shape)
            dtype = _mb.dt.np(alloc.dtype)
            out_avals.append(jax.core.ShapedArray(shape, dtype))
            zero_outs.append(np.zeros(shape, dtype))
    n_params = len(in_names)
    all_in_names = in_names + out_names
    if partition_name is not None:
        all_in_names.append(partition_name)

    def _body(*args):
        operands = list(args)
        if partition_name is not None:
            operands.append(b2j.partition_id_tensor())
        outs = b2j._bass_exec_p.bind(
            *operands,
            out_avals=tuple(out_avals),
            in_names=tuple(all_in_names),
            out_names=tuple(out_names),
            lowering_input_output_aliases=(),
            sim_require_finite=True,
            sim_require_nnan=True,
            nc=nc,
        )
        return tuple(outs)

    devices = jax.devices()[:n_cores]
    mesh = Mesh(np.asarray(devices), ("core",))
    in_specs = (PartitionSpec("core"),) * (n_params + len(out_names))
    out_specs = (PartitionSpec("core"),) * len(out_names)
    donate = tuple(range(n_params, n_params + len(out_names)))
    sharded = jax.jit(
        shard_map(
            _body, mesh=mesh, in_specs=in_specs, out_specs=out_specs, check_rep=False
        ),
        donate_argnums=donate,
        keep_unused=True,
    )

    sharding = jax.sharding.NamedSharding(mesh, PartitionSpec("core"))
    dev_in = [
        jax.device_put(
            np.concatenate([np.asarray(m[name]) for m in in_maps], axis=0), sharding
        )
        for name in in_names
    ]

    def call():
        zeros = [np.zeros((n_cores * z.shape[0], *z.shape[1:]), z.dtype) for z in zero_outs]
        outs = sharded(*dev_in, *zeros)
        jax.block_until_ready(outs)
        return outs

    return call, out_names
